# revision 12
# baseline (speedup 1.0000x reference)
"""ExpanderLinear on 8 TRN2 NeuronCores — v2: PE does ONLY matmuls.

y = x @ (weight * mask)^T + bias
  x      [8192, 4096] f32
  weight [4096, 4096] f32
  mask   [4096, 4096] i32 (0/1)
  bias   [4096]       f32
  y      [8192, 4096] f32

Sharding: 2D 4x2 grid — 4 token shards x 2 outdim shards (t_c=o_c=2048 per
core). Each core computes yt = (w*m) @ x^T + b (the TRANSPOSED output tile,
[o_c, t_c]); the host transposes shards during unshard.

Device pipeline (all operand prep OFF the PE):
  x:  SWDGE cast-DMA (f32->bf16) panel loads, then HWDGE xbar DMA-transpose
      into a resident x^T SBUF buffer [128, NP, KT, 128] (k on partitions).
  wm: w f32 loads (ACT queue) + mask i32->f32 cast loads (SWDGE); DVE
      multiply -> bf16; xbar DMA-transpose into wm^T slivers [128, KT, 128].
  PE: pure bf16 matmul stream: stationary = wm^T sliver tile [128k, 128o],
      moving = x^T [128k, 512t], PSUM accumulates f32 over 32 k-tiles.
      No PE transposes -> no HAM-cold restarts, LDWEIGHTS hidden.
  ACT: PSUM->SBUF eviction fused with per-partition bias (Identity act).
  SWDGE: yt stores.
"""
import os
import sys

sys.path.insert(0, "/opt/trn_rl_repo")

import numpy as np  # noqa: E402

import concourse.bass as bass  # noqa: E402,F401
import concourse.mybir as mybir  # noqa: E402
import concourse.tile as tile  # noqa: E402
import concourse.bacc as bacc  # noqa: E402
from concourse.bass_utils import run_bass_kernel_spmd  # noqa: E402
from concourse.bass_interp import get_hw_module  # noqa: E402

TOKENS, INDIM, OUTDIM = 8192, 4096, 4096
R_SHARDS, C_SHARDS = 4, 2
T_C, O_C = TOKENS // R_SHARDS, OUTDIM // C_SHARDS  # 2048, 2048

P = 128      # partitions / k-tile size
TCH = 512    # token chunk (psum free dim)
KPREP = 1024  # wm prep chunk along k


def build_program(t_c=T_C, o_c=O_C, k=INDIM, hw=True):
    KT = k // P           # k-tiles
    NP = t_c // P         # x panels
    NOC = o_c // P        # out tiles
    NTC = t_c // TCH      # token chunks
    NQ = k // KPREP       # wm prep chunks
    PPT = TCH // P        # panels per token chunk
    NJ = KPREP // P       # k-tiles per prep chunk

    nc = bacc.Bacc("TRN2", target_bir_lowering=False, debug=False,
                   num_devices=8)
    x = nc.dram_tensor("x", [t_c, k], mybir.dt.float32, kind="ExternalInput")
    w = nc.dram_tensor("w", [o_c, k], mybir.dt.float32, kind="ExternalInput")
    m = nc.dram_tensor("m", [o_c, k], mybir.dt.int32, kind="ExternalInput")
    b = nc.dram_tensor("b", [o_c], mybir.dt.float32, kind="ExternalInput")
    yt = nc.dram_tensor("yt", [o_c, t_c], mybir.dt.float32,
                        kind="ExternalOutput")

    with tile.TileContext(nc) as tc:
        with (tc.tile_pool(name="xT_pool", bufs=1) as xT_pool,
              tc.tile_pool(name="biasp", bufs=1) as biasp,
              tc.tile_pool(name="xfstage", bufs=2) as xfstage,
              tc.tile_pool(name="xbpan", bufs=2) as xbpan,
              tc.tile_pool(name="wstage", bufs=2) as wstage,
              tc.tile_pool(name="mstage", bufs=2) as mstage,
              tc.tile_pool(name="wmstage", bufs=2) as wmstage,
              tc.tile_pool(name="slivp", bufs=4) as slivp,
              tc.tile_pool(name="outp", bufs=2) as outp,
              tc.tile_pool(name="psum", bufs=8, space="PSUM") as psum_pool):
            xT = xT_pool.tile([P, NP, KT, P], mybir.dt.bfloat16, name="xT")
            bias_sb = biasp.tile([P, NOC], mybir.dt.float32, name="bias_sb")

            for oc in range(NOC):
                nc.gpsimd.dma_start(
                    bias_sb[:, oc:oc + 1],
                    b[oc * P:(oc + 1) * P, None])

            def x_panel(p):
                xb = xbpan.tile([P, k], mybir.dt.bfloat16, tag="xbpan")
                for q in range(max(1, k // KPREP)):
                    cw = min(KPREP, k)
                    k0 = q * cw
                    xf = xfstage.tile([P, cw], mybir.dt.float32, tag="xf")
                    nc.scalar.dma_start(xf[:, :],
                                        x[p * P:(p + 1) * P, k0:k0 + cw])
                    nc.vector.tensor_copy(xb[:, k0:k0 + cw], xf[:, :])
                nc.sync.dma_start(xT[:, p], xb[:, :], transpose=True)

            slivers = {}

            def wm_prep(oc):
                sl = slivp.tile([P, KT, P], mybir.dt.bfloat16, tag="sliv")
                slivers[oc] = sl
                ro = oc * P
                for q in range(NQ):
                    ks = slice(q * KPREP, (q + 1) * KPREP)
                    wch = wstage.tile([P, KPREP], mybir.dt.float32, tag="wch")
                    nc.scalar.dma_start(wch[:, :], w[ro:ro + P, ks])
                    mch = mstage.tile([P, KPREP], mybir.dt.bfloat16, tag="mch")
                    nc.gpsimd.dma_start(mch[:, :], m[ro:ro + P, ks])
                    wmb = wmstage.tile([P, KPREP], mybir.dt.bfloat16,
                                       tag="wmb")
                    nc.vector.tensor_mul(wmb[:, :], wch[:, :], mch[:, :])
                    nc.sync.dma_start(sl[:, q * NJ:(q + 1) * NJ, :],
                                      wmb[:, :], transpose=True)

            def evict(oc, tcx, pt):
                ot = outp.tile([P, TCH], mybir.dt.float32, tag="out")
                nc.vector.tensor_scalar_add(ot[:, :], pt[:, :],
                                            bias_sb[:, oc:oc + 1])
                nc.gpsimd.dma_start(
                    yt[oc * P:(oc + 1) * P, tcx * TCH:(tcx + 1) * TCH],
                    ot[:, :])

            def mm_group(pt, sl, tcx, kt):
                nc.tensor.matmul(
                    pt[:, :], sl[:, kt, :],
                    xT[:, tcx * PPT:(tcx + 1) * PPT, kt, :],
                    start=(kt == 0), stop=(kt == KT - 1))

            # ---- emission ----
            # tc-major ramp: while x panels stream in, process the first
            # RAMP_OC out-tiles against each arriving token chunk; then
            # steady-state k-outer sweeps with sliver prefetch depth 2.
            RAMP_OC = min(4, NOC)
            for tcx in range(NTC):
                for p in range(tcx * PPT, min((tcx + 1) * PPT, NP)):
                    x_panel(p)
                if tcx < min(2, RAMP_OC):
                    wm_prep(tcx)
            for _oc in range(min(2, RAMP_OC)):
                if _oc not in slivers:
                    wm_prep(_oc)

            for tcx in range(NTC):
                last_tc = tcx == NTC - 1
                for j in range(RAMP_OC):
                    if j not in slivers:
                        wm_prep(j)
                    pt = psum_pool.tile([P, TCH], mybir.dt.float32,
                                        tag="acc")
                    for kt in range(KT):
                        mm_group(pt, slivers[j], tcx, kt)
                    evict(j, tcx, pt)
                    if last_tc and RAMP_OC + j < min(RAMP_OC + 2, NOC):
                        wm_prep(RAMP_OC + j)
            for j in range(RAMP_OC):
                slivers.pop(j)
            for _oc in range(RAMP_OC, min(RAMP_OC + 2, NOC)):
                if _oc not in slivers:
                    wm_prep(_oc)

            for oc in range(RAMP_OC, NOC):
                sl = slivers.pop(oc)
                if oc + 2 < NOC:
                    wm_prep(oc + 2)
                pts = []
                for _i in range(NTC):
                    pt = psum_pool.tile([P, TCH], mybir.dt.float32,
                                        tag="acc", name=f"acc{_i}")
                    pts.append(pt)
                for kt in range(KT):
                    for tcx in range(NTC):
                        mm_group(pts[tcx], sl, tcx, kt)
                for tcx in range(NTC):
                    evict(oc, tcx, pts[tcx])

    nc.compile()
    if hw:
        nc.m = get_hw_module(nc.m)
    return nc


_PROGRAM = None


def _get_program():
    global _PROGRAM
    if _PROGRAM is None:
        _PROGRAM = build_program()
    return _PROGRAM


def _enable_tracing():
    """Install the axon NTFF profile hook if the image's antenv lacks it."""
    try:
        import contextlib
        import ctypes
        import types

        import concourse.bass_utils as bu
        bu.upload_artifacts = lambda tmpdir: ""  # no S3 in this container

        try:
            from antenv.axon_hooks import get_axon_ntff_profile_hook
            if get_axon_ntff_profile_hook() is not None:
                return True
        except ImportError:
            pass

        so_path = "/opt/axon/libaxon_pjrt.so"
        if not os.path.exists(so_path):
            return False
        lib = ctypes.CDLL(so_path)
        if not hasattr(lib, "axon_start_nrt_profile"):
            return False
        lib.axon_start_nrt_profile.argtypes = [
            ctypes.POINTER(ctypes.c_int64), ctypes.c_size_t]
        lib.axon_start_nrt_profile.restype = ctypes.c_int64
        lib.axon_stop_nrt_profile.argtypes = [ctypes.c_char_p]
        lib.axon_stop_nrt_profile.restype = ctypes.c_int64

        @contextlib.contextmanager
        def _hook(output_dir, device_ids):
            import jax
            jax.devices()
            if device_ids:
                ids = (ctypes.c_int64 * len(device_ids))(*device_ids)
                rc = lib.axon_start_nrt_profile(ids, len(device_ids))
            else:
                rc = lib.axon_start_nrt_profile(None, 0)
            if rc != 0:
                raise RuntimeError(f"axon_start_nrt_profile rc={rc}")
            try:
                yield
            finally:
                n = lib.axon_stop_nrt_profile(str(output_dir).encode())
                if n <= 0:
                    print(f"ntff profile: rc={n} (no files) -> {output_dir}")

        mod = types.ModuleType("antenv.axon_hooks")
        _state = {"hook": _hook}
        mod.set_axon_ntff_profile_hook = lambda h: _state.update(hook=h)
        mod.get_axon_ntff_profile_hook = lambda: _state["hook"]
        import antenv
        sys.modules["antenv.axon_hooks"] = mod
        antenv.axon_hooks = mod
        return True
    except Exception as e:  # tracing is best-effort
        print(f"tracing unavailable: {e}")
        return False


def kernel(x, weight, bias, mask):
    x = np.asarray(x, dtype=np.float32)
    weight = np.asarray(weight, dtype=np.float32)
    bias = np.asarray(bias, dtype=np.float32)
    mask = np.asarray(mask, dtype=np.int32)

    nc = _get_program()

    in_maps = []
    for core in range(8):
        r, c = core // C_SHARDS, core % C_SHARDS
        in_maps.append({
            "x": np.ascontiguousarray(x[r * T_C:(r + 1) * T_C]),
            "w": np.ascontiguousarray(weight[c * O_C:(c + 1) * O_C]),
            "m": np.ascontiguousarray(mask[c * O_C:(c + 1) * O_C]),
            "b": np.ascontiguousarray(bias[c * O_C:(c + 1) * O_C]),
        })

    trace = os.environ.get("KERNEL_TRACE", "1") == "1"
    if trace:
        trace = _enable_tracing()
    res = None
    if trace:
        tmpdir = os.environ.get("KERNEL_TRACE_DIR")
        if tmpdir:
            os.makedirs(tmpdir, exist_ok=True)
        try:
            res = run_bass_kernel_spmd(nc, in_maps, core_ids=list(range(8)),
                                       trace=True, tmpdir=tmpdir)
        except Exception as e:
            print(f"traced run failed ({e!r}); rerunning untraced")
            res = None
    if res is None:
        res = run_bass_kernel_spmd(nc, in_maps, core_ids=list(range(8)))
    if res.exec_time_ns is not None:
        print(f"HW exec time: {res.exec_time_ns} ns")

    out = np.empty((TOKENS, OUTDIM), dtype=np.float32)
    for core in range(8):
        r, c = core // C_SHARDS, core % C_SHARDS
        out[r * T_C:(r + 1) * T_C, c * O_C:(c + 1) * O_C] = \
            np.ascontiguousarray(res.results[core]["yt"].T)
    return out


def _sim_test(t_c=512, o_c=256, k=1024):
    """CoreSim numerics check at reduced size."""
    from concourse.bass_interp import CoreSim
    rng = np.random.default_rng(0)
    xv = rng.standard_normal((t_c, k), dtype=np.float32)
    wv = rng.standard_normal((o_c, k), dtype=np.float32) * 0.03
    mv = rng.integers(0, 2, size=(o_c, k)).astype(np.int32)
    bv = rng.standard_normal(o_c).astype(np.float32)

    nc = build_program(t_c=t_c, o_c=o_c, k=k, hw=False)
    sim = CoreSim(nc)
    sim.tensor("x")[:] = xv
    sim.tensor("w")[:] = wv
    sim.tensor("m")[:] = mv
    sim.tensor("b")[:] = bv
    sim.simulate(check_with_hw=False)
    got = np.array(sim.tensor("yt")).T  # [t_c, o_c]

    wm = wv * mv
    ref = xv @ wm.T + bv
    num = np.linalg.norm((got - ref).astype(np.float64))
    den = np.linalg.norm(ref.astype(np.float64)) + 1e-30
    print(f"sim rel err: {num / den:.6g}  (max abs {np.abs(got - ref).max():.4g})")
    assert num / den < 2e-2, "sim numerics check FAILED"
    print("SIM OK")


if __name__ == "__main__":
    _sim_test()


# revision 14
# speedup vs baseline: 1.1193x; 1.1193x over previous
"""ExpanderLinear on 8 TRN2 NeuronCores — v2: PE does ONLY matmuls.

y = x @ (weight * mask)^T + bias
  x      [8192, 4096] f32
  weight [4096, 4096] f32
  mask   [4096, 4096] i32 (0/1)
  bias   [4096]       f32
  y      [8192, 4096] f32

Sharding: 2D 4x2 grid — 4 token shards x 2 outdim shards (t_c=o_c=2048 per
core). Each core computes yt = (w*m) @ x^T + b (the TRANSPOSED output tile,
[o_c, t_c]); the host transposes shards during unshard.

Device pipeline (all operand prep OFF the PE):
  x:  SWDGE cast-DMA (f32->bf16) panel loads, then HWDGE xbar DMA-transpose
      into a resident x^T SBUF buffer [128, NP, KT, 128] (k on partitions).
  wm: w f32 loads (ACT queue) + mask i32->f32 cast loads (SWDGE); DVE
      multiply -> bf16; xbar DMA-transpose into wm^T slivers [128, KT, 128].
  PE: pure bf16 matmul stream: stationary = wm^T sliver tile [128k, 128o],
      moving = x^T [128k, 512t], PSUM accumulates f32 over 32 k-tiles.
      No PE transposes -> no HAM-cold restarts, LDWEIGHTS hidden.
  ACT: PSUM->SBUF eviction fused with per-partition bias (Identity act).
  SWDGE: yt stores.
"""
import os
import sys

sys.path.insert(0, "/opt/trn_rl_repo")

import numpy as np  # noqa: E402

import concourse.bass as bass  # noqa: E402,F401
import concourse.mybir as mybir  # noqa: E402
import concourse.tile as tile  # noqa: E402
import concourse.bacc as bacc  # noqa: E402
from concourse.bass_utils import run_bass_kernel_spmd  # noqa: E402
from concourse.bass_interp import get_hw_module  # noqa: E402

TOKENS, INDIM, OUTDIM = 8192, 4096, 4096
R_SHARDS, C_SHARDS = 4, 2
T_C, O_C = TOKENS // R_SHARDS, OUTDIM // C_SHARDS  # 2048, 2048

P = 128      # partitions / k-tile size
TCH = 512    # token chunk (psum free dim)
KPREP = 1024  # wm prep chunk along k


def build_program(t_c=T_C, o_c=O_C, k=INDIM, hw=True):
    KT = k // P           # k-tiles
    NP = t_c // P         # x panels
    NOC = o_c // P        # out tiles
    NTC = t_c // TCH      # token chunks
    NQ = k // KPREP       # wm prep chunks
    PPT = TCH // P        # panels per token chunk
    NJ = KPREP // P       # k-tiles per prep chunk

    nc = bacc.Bacc("TRN2", target_bir_lowering=False, debug=False,
                   num_devices=8)
    x = nc.dram_tensor("x", [t_c, k], mybir.dt.float32, kind="ExternalInput")
    w = nc.dram_tensor("w", [o_c, k], mybir.dt.float32, kind="ExternalInput")
    m = nc.dram_tensor("m", [o_c, k], mybir.dt.int32, kind="ExternalInput")
    b = nc.dram_tensor("b", [o_c], mybir.dt.float32, kind="ExternalInput")
    yt = nc.dram_tensor("yt", [o_c, t_c], mybir.dt.float32,
                        kind="ExternalOutput")

    with tile.TileContext(nc) as tc:
        with (tc.tile_pool(name="xT_pool", bufs=1) as xT_pool,
              tc.tile_pool(name="biasp", bufs=1) as biasp,
              tc.tile_pool(name="xfstage", bufs=4) as xfstage,
              tc.tile_pool(name="xbpan", bufs=2) as xbpan,
              tc.tile_pool(name="wstage", bufs=2) as wstage,
              tc.tile_pool(name="mstage", bufs=2) as mstage,
              tc.tile_pool(name="wmstage", bufs=2) as wmstage,
              tc.tile_pool(name="slivp", bufs=3) as slivp,
              tc.tile_pool(name="outp", bufs=3) as outp,
              tc.tile_pool(name="psum", bufs=8, space="PSUM") as psum_pool):
            xT = xT_pool.tile([P, NP, KT, P], mybir.dt.bfloat16, name="xT")
            bias_sb = biasp.tile([P, NOC], mybir.dt.float32, name="bias_sb")

            for oc in range(NOC):
                nc.gpsimd.dma_start(
                    bias_sb[:, oc:oc + 1],
                    b[oc * P:(oc + 1) * P, None])

            def x_panel(p):
                xb = xbpan.tile([P, k], mybir.dt.bfloat16, tag="xbpan")
                for q in range(max(1, k // KPREP)):
                    cw = min(KPREP, k)
                    k0 = q * cw
                    xf = xfstage.tile([P, cw], mybir.dt.float32, tag="xf")
                    nc.scalar.dma_start(xf[:, :],
                                        x[p * P:(p + 1) * P, k0:k0 + cw])
                    nc.vector.tensor_copy(xb[:, k0:k0 + cw], xf[:, :])
                nc.sync.dma_start(xT[:, p], xb[:, :], transpose=True)

            slivers = {}

            def wm_prep(oc):
                sl = slivp.tile([P, KT, P], mybir.dt.bfloat16, tag="sliv")
                slivers[oc] = sl
                ro = oc * P
                for q in range(NQ):
                    ks = slice(q * KPREP, (q + 1) * KPREP)
                    wch = wstage.tile([P, KPREP], mybir.dt.float32, tag="wch")
                    nc.scalar.dma_start(wch[:, :], w[ro:ro + P, ks])
                    mch = mstage.tile([P, KPREP], mybir.dt.bfloat16, tag="mch")
                    nc.gpsimd.dma_start(mch[:, :], m[ro:ro + P, ks])
                    wmb = wmstage.tile([P, KPREP], mybir.dt.bfloat16,
                                       tag="wmb")
                    nc.vector.tensor_mul(wmb[:, :], wch[:, :], mch[:, :])
                    nc.sync.dma_start(sl[:, q * NJ:(q + 1) * NJ, :],
                                      wmb[:, :], transpose=True)

            def evict(oc, tcx, pt):
                ot = outp.tile([P, TCH], mybir.dt.float32, tag="out")
                nc.vector.tensor_scalar_add(ot[:, :], pt[:, :],
                                            bias_sb[:, oc:oc + 1])
                nc.gpsimd.dma_start(
                    yt[oc * P:(oc + 1) * P, tcx * TCH:(tcx + 1) * TCH],
                    ot[:, :])

            def mm_group(pt, sl, tcx, kt):
                nc.tensor.matmul(
                    pt[:, :], sl[:, kt, :],
                    xT[:, tcx * PPT:(tcx + 1) * PPT, kt, :],
                    start=(kt == 0), stop=(kt == KT - 1))

            # ---- emission ----
            for tcx in range(NTC):
                for p in range(tcx * PPT, min((tcx + 1) * PPT, NP)):
                    x_panel(p)
                if tcx < min(2, NOC):
                    wm_prep(tcx)
            for _oc in range(min(3, NOC)):
                if _oc not in slivers:
                    wm_prep(_oc)

            for oc in range(NOC):
                sl = slivers.pop(oc)
                if oc == 0:
                    # tc-outer: follow x panel arrival order
                    for tcx in range(NTC):
                        pt = psum_pool.tile([P, TCH], mybir.dt.float32,
                                            tag="acc")
                        for kt in range(KT):
                            mm_group(pt, sl, tcx, kt)
                        evict(oc, tcx, pt)
                else:
                    if oc + 2 < NOC:
                        wm_prep(oc + 2)
                    pts = []
                    for _i in range(NTC):
                        pt = psum_pool.tile([P, TCH], mybir.dt.float32,
                                            tag="acc", name=f"acc{_i}")
                        pts.append(pt)
                    for kt in range(KT):
                        for tcx in range(NTC):
                            mm_group(pts[tcx], sl, tcx, kt)
                    for tcx in range(NTC):
                        evict(oc, tcx, pts[tcx])

    nc.compile()
    if hw:
        nc.m = get_hw_module(nc.m)
    return nc


_PROGRAM = None


def _get_program():
    global _PROGRAM
    if _PROGRAM is None:
        _PROGRAM = build_program()
    return _PROGRAM


def _enable_tracing():
    """Install the axon NTFF profile hook if the image's antenv lacks it."""
    try:
        import contextlib
        import ctypes
        import types

        import concourse.bass_utils as bu
        bu.upload_artifacts = lambda tmpdir: ""  # no S3 in this container

        try:
            from antenv.axon_hooks import get_axon_ntff_profile_hook
            if get_axon_ntff_profile_hook() is not None:
                return True
        except ImportError:
            pass

        so_path = "/opt/axon/libaxon_pjrt.so"
        if not os.path.exists(so_path):
            return False
        lib = ctypes.CDLL(so_path)
        if not hasattr(lib, "axon_start_nrt_profile"):
            return False
        lib.axon_start_nrt_profile.argtypes = [
            ctypes.POINTER(ctypes.c_int64), ctypes.c_size_t]
        lib.axon_start_nrt_profile.restype = ctypes.c_int64
        lib.axon_stop_nrt_profile.argtypes = [ctypes.c_char_p]
        lib.axon_stop_nrt_profile.restype = ctypes.c_int64

        @contextlib.contextmanager
        def _hook(output_dir, device_ids):
            import jax
            jax.devices()
            if device_ids:
                ids = (ctypes.c_int64 * len(device_ids))(*device_ids)
                rc = lib.axon_start_nrt_profile(ids, len(device_ids))
            else:
                rc = lib.axon_start_nrt_profile(None, 0)
            if rc != 0:
                raise RuntimeError(f"axon_start_nrt_profile rc={rc}")
            try:
                yield
            finally:
                n = lib.axon_stop_nrt_profile(str(output_dir).encode())
                if n <= 0:
                    print(f"ntff profile: rc={n} (no files) -> {output_dir}")

        mod = types.ModuleType("antenv.axon_hooks")
        _state = {"hook": _hook}
        mod.set_axon_ntff_profile_hook = lambda h: _state.update(hook=h)
        mod.get_axon_ntff_profile_hook = lambda: _state["hook"]
        import antenv
        sys.modules["antenv.axon_hooks"] = mod
        antenv.axon_hooks = mod
        return True
    except Exception as e:  # tracing is best-effort
        print(f"tracing unavailable: {e}")
        return False


def kernel(x, weight, bias, mask):
    x = np.asarray(x, dtype=np.float32)
    weight = np.asarray(weight, dtype=np.float32)
    bias = np.asarray(bias, dtype=np.float32)
    mask = np.asarray(mask, dtype=np.int32)

    nc = _get_program()

    in_maps = []
    for core in range(8):
        r, c = core // C_SHARDS, core % C_SHARDS
        in_maps.append({
            "x": np.ascontiguousarray(x[r * T_C:(r + 1) * T_C]),
            "w": np.ascontiguousarray(weight[c * O_C:(c + 1) * O_C]),
            "m": np.ascontiguousarray(mask[c * O_C:(c + 1) * O_C]),
            "b": np.ascontiguousarray(bias[c * O_C:(c + 1) * O_C]),
        })

    trace = os.environ.get("KERNEL_TRACE", "1") == "1"
    if trace:
        trace = _enable_tracing()
    res = None
    if trace:
        tmpdir = os.environ.get("KERNEL_TRACE_DIR")
        if tmpdir:
            os.makedirs(tmpdir, exist_ok=True)
        try:
            res = run_bass_kernel_spmd(nc, in_maps, core_ids=list(range(8)),
                                       trace=True, tmpdir=tmpdir)
        except Exception as e:
            print(f"traced run failed ({e!r}); rerunning untraced")
            res = None
    if res is None:
        res = run_bass_kernel_spmd(nc, in_maps, core_ids=list(range(8)))
    if res.exec_time_ns is not None:
        print(f"HW exec time: {res.exec_time_ns} ns")

    out = np.empty((TOKENS, OUTDIM), dtype=np.float32)
    for core in range(8):
        r, c = core // C_SHARDS, core % C_SHARDS
        out[r * T_C:(r + 1) * T_C, c * O_C:(c + 1) * O_C] = \
            np.ascontiguousarray(res.results[core]["yt"].T)
    return out


def _sim_test(t_c=512, o_c=256, k=1024):
    """CoreSim numerics check at reduced size."""
    from concourse.bass_interp import CoreSim
    rng = np.random.default_rng(0)
    xv = rng.standard_normal((t_c, k), dtype=np.float32)
    wv = rng.standard_normal((o_c, k), dtype=np.float32) * 0.03
    mv = rng.integers(0, 2, size=(o_c, k)).astype(np.int32)
    bv = rng.standard_normal(o_c).astype(np.float32)

    nc = build_program(t_c=t_c, o_c=o_c, k=k, hw=False)
    sim = CoreSim(nc)
    sim.tensor("x")[:] = xv
    sim.tensor("w")[:] = wv
    sim.tensor("m")[:] = mv
    sim.tensor("b")[:] = bv
    sim.simulate(check_with_hw=False)
    got = np.array(sim.tensor("yt")).T  # [t_c, o_c]

    wm = wv * mv
    ref = xv @ wm.T + bv
    num = np.linalg.norm((got - ref).astype(np.float64))
    den = np.linalg.norm(ref.astype(np.float64)) + 1e-30
    print(f"sim rel err: {num / den:.6g}  (max abs {np.abs(got - ref).max():.4g})")
    assert num / den < 2e-2, "sim numerics check FAILED"
    print("SIM OK")


if __name__ == "__main__":
    _sim_test()


# revision 15
# speedup vs baseline: 1.1359x; 1.0149x over previous
"""ExpanderLinear on 8 TRN2 NeuronCores — v2: PE does ONLY matmuls.

y = x @ (weight * mask)^T + bias
  x      [8192, 4096] f32
  weight [4096, 4096] f32
  mask   [4096, 4096] i32 (0/1)
  bias   [4096]       f32
  y      [8192, 4096] f32

Sharding: 2D 4x2 grid — 4 token shards x 2 outdim shards (t_c=o_c=2048 per
core). Each core computes yt = (w*m) @ x^T + b (the TRANSPOSED output tile,
[o_c, t_c]); the host transposes shards during unshard.

Device pipeline (all operand prep OFF the PE):
  x:  raw f32 quarter-loads (ACT HWDGE ring) -> DVE cast to bf16 panels ->
      HWDGE xbar DMA-transpose into a resident x^T SBUF buffer
      [128, NP, KT, 128] (k on partitions).
  wm: w f32 loads (ACT queue) + mask i32->bf16 cast loads (SWDGE); DVE
      multiply -> bf16; xbar DMA-transpose into wm^T slivers [128, KT, 128].
  PE: pure bf16 matmul stream: stationary = wm^T sliver tile [128k, 128o],
      moving = x^T [128k, 512t], PSUM accumulates f32 over 32 k-tiles.
      No PE transposes -> no HAM-cold restarts, LDWEIGHTS hidden.
  DVE: PSUM->SBUF eviction fused with per-partition bias (tensor_scalar).
  SWDGE: yt stores.
"""
import os
import sys

sys.path.insert(0, "/opt/trn_rl_repo")

import numpy as np  # noqa: E402

import concourse.bass as bass  # noqa: E402,F401
import concourse.mybir as mybir  # noqa: E402
import concourse.tile as tile  # noqa: E402
import concourse.bacc as bacc  # noqa: E402
from concourse.bass_utils import run_bass_kernel_spmd  # noqa: E402
from concourse.bass_interp import get_hw_module  # noqa: E402

TOKENS, INDIM, OUTDIM = 8192, 4096, 4096
R_SHARDS, C_SHARDS = 4, 2
T_C, O_C = TOKENS // R_SHARDS, OUTDIM // C_SHARDS  # 2048, 2048

P = 128      # partitions / k-tile size
TCH = 512    # token chunk (psum free dim)
KPREP = 1024  # wm prep chunk along k


def build_program(t_c=T_C, o_c=O_C, k=INDIM, hw=True):
    KT = k // P           # k-tiles
    NP = t_c // P         # x panels
    NOC = o_c // P        # out tiles
    NTC = t_c // TCH      # token chunks
    NQ = k // KPREP       # wm prep chunks
    PPT = TCH // P        # panels per token chunk
    NJ = KPREP // P       # k-tiles per prep chunk

    nc = bacc.Bacc("TRN2", target_bir_lowering=False, debug=False,
                   num_devices=8)
    x = nc.dram_tensor("x", [t_c, k], mybir.dt.float32, kind="ExternalInput")
    w = nc.dram_tensor("w", [o_c, k], mybir.dt.float32, kind="ExternalInput")
    m = nc.dram_tensor("m", [o_c, k], mybir.dt.int32, kind="ExternalInput")
    b = nc.dram_tensor("b", [o_c], mybir.dt.float32, kind="ExternalInput")
    yt = nc.dram_tensor("yt", [o_c, t_c], mybir.dt.float32,
                        kind="ExternalOutput")

    with tile.TileContext(nc) as tc:
        with (tc.tile_pool(name="xT_pool", bufs=1) as xT_pool,
              tc.tile_pool(name="biasp", bufs=1) as biasp,
              tc.tile_pool(name="xfstage", bufs=4) as xfstage,
              tc.tile_pool(name="xbpan", bufs=2) as xbpan,
              tc.tile_pool(name="wstage", bufs=3) as wstage,
              tc.tile_pool(name="mstage", bufs=3) as mstage,
              tc.tile_pool(name="wmstage", bufs=3) as wmstage,
              tc.tile_pool(name="slivp", bufs=2) as slivp,
              tc.tile_pool(name="outp", bufs=3) as outp,
              tc.tile_pool(name="psum", bufs=8, space="PSUM") as psum_pool):
            xT = xT_pool.tile([P, NP, KT, P], mybir.dt.bfloat16, name="xT")
            bias_sb = biasp.tile([P, NOC], mybir.dt.float32, name="bias_sb")

            for oc in range(NOC):
                nc.gpsimd.dma_start(
                    bias_sb[:, oc:oc + 1],
                    b[oc * P:(oc + 1) * P, None])

            def x_panel(p):
                xb = xbpan.tile([P, k], mybir.dt.bfloat16, tag="xbpan")
                for q in range(max(1, k // KPREP)):
                    cw = min(KPREP, k)
                    k0 = q * cw
                    xf = xfstage.tile([P, cw], mybir.dt.float32, tag="xf")
                    nc.scalar.dma_start(xf[:, :],
                                        x[p * P:(p + 1) * P, k0:k0 + cw])
                    nc.vector.tensor_copy(xb[:, k0:k0 + cw], xf[:, :])
                nc.sync.dma_start(xT[:, p], xb[:, :], transpose=True)

            slivers = {}

            def wm_prep(oc):
                sl = slivp.tile([P, KT, P], mybir.dt.bfloat16, tag="sliv")
                slivers[oc] = sl
                ro = oc * P
                for q in range(NQ):
                    ks = slice(q * KPREP, (q + 1) * KPREP)
                    wch = wstage.tile([P, KPREP], mybir.dt.float32, tag="wch")
                    nc.scalar.dma_start(wch[:, :], w[ro:ro + P, ks])
                    mch = mstage.tile([P, KPREP], mybir.dt.bfloat16, tag="mch")
                    nc.gpsimd.dma_start(mch[:, :], m[ro:ro + P, ks])
                    wmb = wmstage.tile([P, KPREP], mybir.dt.bfloat16,
                                       tag="wmb")
                    nc.vector.tensor_mul(wmb[:, :], wch[:, :], mch[:, :])
                    nc.sync.dma_start(sl[:, q * NJ:(q + 1) * NJ, :],
                                      wmb[:, :], transpose=True)

            def evict(oc, tcx, pt):
                ot = outp.tile([P, TCH], mybir.dt.float32, tag="out")
                nc.vector.tensor_scalar_add(ot[:, :], pt[:, :],
                                            bias_sb[:, oc:oc + 1])
                nc.gpsimd.dma_start(
                    yt[oc * P:(oc + 1) * P, tcx * TCH:(tcx + 1) * TCH],
                    ot[:, :])

            def mm_group(pt, sl, tcx, kt):
                nc.tensor.matmul(
                    pt[:, :], sl[:, kt, :],
                    xT[:, tcx * PPT:(tcx + 1) * PPT, kt, :],
                    start=(kt == 0), stop=(kt == KT - 1))

            # ---- emission ----
            for tcx in range(NTC):
                for p in range(tcx * PPT, min((tcx + 1) * PPT, NP)):
                    x_panel(p)
                if tcx < min(2, NOC):
                    wm_prep(tcx)
            for _oc in range(min(2, NOC)):
                if _oc not in slivers:
                    wm_prep(_oc)

            for oc in range(NOC):
                sl = slivers.pop(oc)
                if oc == 0:
                    # tc-outer: follow x panel arrival order
                    for tcx in range(NTC):
                        pt = psum_pool.tile([P, TCH], mybir.dt.float32,
                                            tag="acc")
                        for kt in range(KT):
                            mm_group(pt, sl, tcx, kt)
                        evict(oc, tcx, pt)
                else:
                    if oc + 1 < NOC:
                        wm_prep(oc + 1)
                    pts = []
                    for _i in range(NTC):
                        pt = psum_pool.tile([P, TCH], mybir.dt.float32,
                                            tag="acc", name=f"acc{_i}")
                        pts.append(pt)
                    for kt in range(KT):
                        for tcx in range(NTC):
                            mm_group(pts[tcx], sl, tcx, kt)
                    for tcx in range(NTC):
                        evict(oc, tcx, pts[tcx])

    nc.compile()
    if hw:
        nc.m = get_hw_module(nc.m)
    return nc


_PROGRAM = None


def _get_program():
    global _PROGRAM
    if _PROGRAM is None:
        _PROGRAM = build_program()
    return _PROGRAM


def _enable_tracing():
    """Install the axon NTFF profile hook if the image's antenv lacks it."""
    try:
        import contextlib
        import ctypes
        import types

        import concourse.bass_utils as bu
        bu.upload_artifacts = lambda tmpdir: ""  # no S3 in this container

        try:
            from antenv.axon_hooks import get_axon_ntff_profile_hook
            if get_axon_ntff_profile_hook() is not None:
                return True
        except ImportError:
            pass

        so_path = "/opt/axon/libaxon_pjrt.so"
        if not os.path.exists(so_path):
            return False
        lib = ctypes.CDLL(so_path)
        if not hasattr(lib, "axon_start_nrt_profile"):
            return False
        lib.axon_start_nrt_profile.argtypes = [
            ctypes.POINTER(ctypes.c_int64), ctypes.c_size_t]
        lib.axon_start_nrt_profile.restype = ctypes.c_int64
        lib.axon_stop_nrt_profile.argtypes = [ctypes.c_char_p]
        lib.axon_stop_nrt_profile.restype = ctypes.c_int64

        @contextlib.contextmanager
        def _hook(output_dir, device_ids):
            import jax
            jax.devices()
            if device_ids:
                ids = (ctypes.c_int64 * len(device_ids))(*device_ids)
                rc = lib.axon_start_nrt_profile(ids, len(device_ids))
            else:
                rc = lib.axon_start_nrt_profile(None, 0)
            if rc != 0:
                raise RuntimeError(f"axon_start_nrt_profile rc={rc}")
            try:
                yield
            finally:
                n = lib.axon_stop_nrt_profile(str(output_dir).encode())
                if n <= 0:
                    print(f"ntff profile: rc={n} (no files) -> {output_dir}")

        mod = types.ModuleType("antenv.axon_hooks")
        _state = {"hook": _hook}
        mod.set_axon_ntff_profile_hook = lambda h: _state.update(hook=h)
        mod.get_axon_ntff_profile_hook = lambda: _state["hook"]
        import antenv
        sys.modules["antenv.axon_hooks"] = mod
        antenv.axon_hooks = mod
        return True
    except Exception as e:  # tracing is best-effort
        print(f"tracing unavailable: {e}")
        return False


def kernel(x, weight, bias, mask):
    x = np.asarray(x, dtype=np.float32)
    weight = np.asarray(weight, dtype=np.float32)
    bias = np.asarray(bias, dtype=np.float32)
    mask = np.asarray(mask, dtype=np.int32)

    nc = _get_program()

    in_maps = []
    for core in range(8):
        r, c = core // C_SHARDS, core % C_SHARDS
        in_maps.append({
            "x": np.ascontiguousarray(x[r * T_C:(r + 1) * T_C]),
            "w": np.ascontiguousarray(weight[c * O_C:(c + 1) * O_C]),
            "m": np.ascontiguousarray(mask[c * O_C:(c + 1) * O_C]),
            "b": np.ascontiguousarray(bias[c * O_C:(c + 1) * O_C]),
        })

    trace = os.environ.get("KERNEL_TRACE", "1") == "1"
    if trace:
        trace = _enable_tracing()
    res = None
    if trace:
        tmpdir = os.environ.get("KERNEL_TRACE_DIR")
        if tmpdir:
            os.makedirs(tmpdir, exist_ok=True)
        try:
            res = run_bass_kernel_spmd(nc, in_maps, core_ids=list(range(8)),
                                       trace=True, tmpdir=tmpdir)
        except Exception as e:
            print(f"traced run failed ({e!r}); rerunning untraced")
            res = None
    if res is None:
        res = run_bass_kernel_spmd(nc, in_maps, core_ids=list(range(8)))
    if res.exec_time_ns is not None:
        print(f"HW exec time: {res.exec_time_ns} ns")

    out = np.empty((TOKENS, OUTDIM), dtype=np.float32)
    for core in range(8):
        r, c = core // C_SHARDS, core % C_SHARDS
        out[r * T_C:(r + 1) * T_C, c * O_C:(c + 1) * O_C] = \
            np.ascontiguousarray(res.results[core]["yt"].T)
    return out


def _sim_test(t_c=512, o_c=256, k=1024):
    """CoreSim numerics check at reduced size."""
    from concourse.bass_interp import CoreSim
    rng = np.random.default_rng(0)
    xv = rng.standard_normal((t_c, k), dtype=np.float32)
    wv = rng.standard_normal((o_c, k), dtype=np.float32) * 0.03
    mv = rng.integers(0, 2, size=(o_c, k)).astype(np.int32)
    bv = rng.standard_normal(o_c).astype(np.float32)

    nc = build_program(t_c=t_c, o_c=o_c, k=k, hw=False)
    sim = CoreSim(nc)
    sim.tensor("x")[:] = xv
    sim.tensor("w")[:] = wv
    sim.tensor("m")[:] = mv
    sim.tensor("b")[:] = bv
    sim.simulate(check_with_hw=False)
    got = np.array(sim.tensor("yt")).T  # [t_c, o_c]

    wm = wv * mv
    ref = xv @ wm.T + bv
    num = np.linalg.norm((got - ref).astype(np.float64))
    den = np.linalg.norm(ref.astype(np.float64)) + 1e-30
    print(f"sim rel err: {num / den:.6g}  (max abs {np.abs(got - ref).max():.4g})")
    assert num / den < 2e-2, "sim numerics check FAILED"
    print("SIM OK")


if __name__ == "__main__":
    _sim_test()


# revision 16
# speedup vs baseline: 1.3512x; 1.1895x over previous
"""ExpanderLinear on 8 TRN2 NeuronCores — v2: PE does ONLY matmuls.

y = x @ (weight * mask)^T + bias
  x      [8192, 4096] f32
  weight [4096, 4096] f32
  mask   [4096, 4096] i32 (0/1)
  bias   [4096]       f32
  y      [8192, 4096] f32

Sharding: 2D 4x2 grid — 4 token shards x 2 outdim shards (t_c=o_c=2048 per
core). Each core computes yt = (w*m) @ x^T + b (the TRANSPOSED output tile,
[o_c, t_c]); the host transposes shards during unshard.

Device pipeline (all operand prep OFF the PE):
  x:  raw f32 quarter-loads (ACT HWDGE ring) -> DVE cast to bf16 panels ->
      HWDGE xbar DMA-transpose into a resident x^T SBUF buffer
      [128, NP, KT, 128] (k on partitions).
  wm: w f32 loads (ACT queue) + mask i32->bf16 cast loads (SWDGE); DVE
      multiply -> bf16; xbar DMA-transpose into wm^T slivers [128, KT, 128].
  PE: pure bf16 matmul stream: stationary = wm^T sliver tile [128k, 128o],
      moving = x^T [128k, 512t], PSUM accumulates f32 over 32 k-tiles.
      No PE transposes -> no HAM-cold restarts, LDWEIGHTS hidden.
  DVE: PSUM->SBUF eviction fused with per-partition bias (tensor_scalar).
  SWDGE: yt stores.
"""
import os
import sys

sys.path.insert(0, "/opt/trn_rl_repo")

import numpy as np  # noqa: E402

import concourse.bass as bass  # noqa: E402,F401
import concourse.mybir as mybir  # noqa: E402
import concourse.tile as tile  # noqa: E402
import concourse.bacc as bacc  # noqa: E402
from concourse.bass_utils import run_bass_kernel_spmd  # noqa: E402
from concourse.bass_interp import get_hw_module  # noqa: E402

TOKENS, INDIM, OUTDIM = 8192, 4096, 4096
R_SHARDS, C_SHARDS = 4, 2
T_C, O_C = TOKENS // R_SHARDS, OUTDIM // C_SHARDS  # 2048, 2048

P = 128      # partitions / k-tile size
TCH = 512    # token chunk (psum free dim)
KPREP = 1024  # wm prep chunk along k


def build_program(t_c=T_C, o_c=O_C, k=INDIM, hw=True):
    KT = k // P           # k-tiles
    NP = t_c // P         # x panels
    NOC = o_c // P        # out tiles
    NTC = t_c // TCH      # token chunks
    NQ = k // KPREP       # wm prep chunks
    PPT = TCH // P        # panels per token chunk
    NJ = KPREP // P       # k-tiles per prep chunk

    nc = bacc.Bacc("TRN2", target_bir_lowering=False, debug=False,
                   num_devices=8)
    x = nc.dram_tensor("x", [t_c, k], mybir.dt.float32, kind="ExternalInput")
    w = nc.dram_tensor("w", [o_c, k], mybir.dt.float32, kind="ExternalInput")
    m = nc.dram_tensor("m", [o_c, k], mybir.dt.int32, kind="ExternalInput")
    b = nc.dram_tensor("b", [o_c], mybir.dt.float32, kind="ExternalInput")
    yt = nc.dram_tensor("yt", [o_c, t_c], mybir.dt.float32,
                        kind="ExternalOutput")

    with tile.TileContext(nc) as tc:
        with (tc.tile_pool(name="xT_pool", bufs=1) as xT_pool,
              tc.tile_pool(name="biasp", bufs=1) as biasp,
              tc.tile_pool(name="xfstage", bufs=4) as xfstage,
              tc.tile_pool(name="xbpan", bufs=2) as xbpan,
              tc.tile_pool(name="wstage", bufs=3) as wstage,
              tc.tile_pool(name="mstage", bufs=3) as mstage,
              tc.tile_pool(name="wmstage", bufs=3) as wmstage,
              tc.tile_pool(name="slivp", bufs=2) as slivp,
              tc.tile_pool(name="outp", bufs=3) as outp,
              tc.tile_pool(name="consts", bufs=1) as consts,
              tc.tile_pool(name="psum", bufs=6, space="PSUM") as psum_pool,
              tc.tile_pool(name="psum_t", bufs=2, space="PSUM") as psum_t):
            from concourse.masks import make_identity
            xT = xT_pool.tile([P, KT, t_c], mybir.dt.bfloat16, name="xT")
            ident = consts.tile([P, P], mybir.dt.bfloat16, name="ident")
            make_identity(nc, ident)
            bias_sb = biasp.tile([P, NOC], mybir.dt.float32, name="bias_sb")

            for oc in range(NOC):
                nc.gpsimd.dma_start(
                    bias_sb[:, oc:oc + 1],
                    b[oc * P:(oc + 1) * P, None])

            TG = min(8, KT)  # k-tiles per transpose-psum group
            xpan = {}

            def x_panel(p):
                xb = xbpan.tile([P, k], mybir.dt.bfloat16, tag="xbpan")
                xpan[p] = xb
                for q in range(max(1, k // KPREP)):
                    cw = min(KPREP, k)
                    k0 = q * cw
                    xf = xfstage.tile([P, cw], mybir.dt.float32, tag="xf")
                    nc.scalar.dma_start(xf[:, :],
                                        x[p * P:(p + 1) * P, k0:k0 + cw])
                    nc.vector.tensor_copy(xb[:, k0:k0 + cw], xf[:, :])

            def x_tr(p):
                # PE transposes the bf16 panel into resident x^T
                xb = xpan.pop(p)
                for g in range(KT // TG):
                    pt = psum_t.tile([P, TG, P], mybir.dt.bfloat16,
                                     tag="tpsum")
                    for j in range(TG):
                        kt = g * TG + j
                        nc.tensor.transpose(pt[:, j, :],
                                            xb[:, kt * P:(kt + 1) * P],
                                            ident[:, :])
                    nc.vector.tensor_copy(
                        xT[:, g * TG:(g + 1) * TG, p * P:(p + 1) * P],
                        pt[:, :, :])

            slivers = {}

            def wm_prep(oc):
                sl = slivp.tile([P, KT, P], mybir.dt.bfloat16, tag="sliv")
                slivers[oc] = sl
                ro = oc * P
                for q in range(NQ):
                    ks = slice(q * KPREP, (q + 1) * KPREP)
                    wch = wstage.tile([P, KPREP], mybir.dt.float32, tag="wch")
                    nc.scalar.dma_start(wch[:, :], w[ro:ro + P, ks])
                    mch = mstage.tile([P, KPREP], mybir.dt.bfloat16, tag="mch")
                    nc.gpsimd.dma_start(mch[:, :], m[ro:ro + P, ks])
                    wmb = wmstage.tile([P, KPREP], mybir.dt.bfloat16,
                                       tag="wmb")
                    nc.vector.tensor_mul(wmb[:, :], wch[:, :], mch[:, :])
                    nc.sync.dma_start(sl[:, q * NJ:(q + 1) * NJ, :],
                                      wmb[:, :], transpose=True)

            def evict(oc, tcx, pt):
                ot = outp.tile([P, TCH], mybir.dt.float32, tag="out")
                nc.vector.tensor_scalar_add(ot[:, :], pt[:, :],
                                            bias_sb[:, oc:oc + 1])
                nc.gpsimd.dma_start(
                    yt[oc * P:(oc + 1) * P, tcx * TCH:(tcx + 1) * TCH],
                    ot[:, :])

            def mm_group(pt, sl, tcx, kt):
                nc.tensor.matmul(
                    pt[:, :], sl[:, kt, :],
                    xT[:, kt, tcx * TCH:(tcx + 1) * TCH],
                    start=(kt == 0), stop=(kt == KT - 1))

            # ---- emission ----
            for tcx in range(NTC):
                for p in range(tcx * PPT, min((tcx + 1) * PPT, NP)):
                    x_panel(p)
                if tcx < min(2, NOC):
                    wm_prep(tcx)
            for _oc in range(min(2, NOC)):
                if _oc not in slivers:
                    wm_prep(_oc)

            for oc in range(NOC):
                sl = slivers.pop(oc)
                if oc == 0:
                    # tc-outer: follow x panel arrival order; PE transposes
                    # each tc-chunk's panels right before its matmul group
                    for tcx in range(NTC):
                        for p in range(tcx * PPT,
                                       min((tcx + 1) * PPT, NP)):
                            x_tr(p)
                        pt = psum_pool.tile([P, TCH], mybir.dt.float32,
                                            tag="acc")
                        for kt in range(KT):
                            mm_group(pt, sl, tcx, kt)
                        evict(oc, tcx, pt)
                else:
                    if oc + 1 < NOC:
                        wm_prep(oc + 1)
                    pts = []
                    for _i in range(NTC):
                        pt = psum_pool.tile([P, TCH], mybir.dt.float32,
                                            tag="acc", name=f"acc{_i}")
                        pts.append(pt)
                    for kt in range(KT):
                        for tcx in range(NTC):
                            mm_group(pts[tcx], sl, tcx, kt)
                    for tcx in range(NTC):
                        evict(oc, tcx, pts[tcx])

    nc.compile()
    if hw:
        nc.m = get_hw_module(nc.m)
    return nc


_PROGRAM = None


def _get_program():
    global _PROGRAM
    if _PROGRAM is None:
        _PROGRAM = build_program()
    return _PROGRAM


def _enable_tracing():
    """Install the axon NTFF profile hook if the image's antenv lacks it."""
    try:
        import contextlib
        import ctypes
        import types

        import concourse.bass_utils as bu
        bu.upload_artifacts = lambda tmpdir: ""  # no S3 in this container

        try:
            from antenv.axon_hooks import get_axon_ntff_profile_hook
            if get_axon_ntff_profile_hook() is not None:
                return True
        except ImportError:
            pass

        so_path = "/opt/axon/libaxon_pjrt.so"
        if not os.path.exists(so_path):
            return False
        lib = ctypes.CDLL(so_path)
        if not hasattr(lib, "axon_start_nrt_profile"):
            return False
        lib.axon_start_nrt_profile.argtypes = [
            ctypes.POINTER(ctypes.c_int64), ctypes.c_size_t]
        lib.axon_start_nrt_profile.restype = ctypes.c_int64
        lib.axon_stop_nrt_profile.argtypes = [ctypes.c_char_p]
        lib.axon_stop_nrt_profile.restype = ctypes.c_int64

        @contextlib.contextmanager
        def _hook(output_dir, device_ids):
            import jax
            jax.devices()
            if device_ids:
                ids = (ctypes.c_int64 * len(device_ids))(*device_ids)
                rc = lib.axon_start_nrt_profile(ids, len(device_ids))
            else:
                rc = lib.axon_start_nrt_profile(None, 0)
            if rc != 0:
                raise RuntimeError(f"axon_start_nrt_profile rc={rc}")
            try:
                yield
            finally:
                n = lib.axon_stop_nrt_profile(str(output_dir).encode())
                if n <= 0:
                    print(f"ntff profile: rc={n} (no files) -> {output_dir}")

        mod = types.ModuleType("antenv.axon_hooks")
        _state = {"hook": _hook}
        mod.set_axon_ntff_profile_hook = lambda h: _state.update(hook=h)
        mod.get_axon_ntff_profile_hook = lambda: _state["hook"]
        import antenv
        sys.modules["antenv.axon_hooks"] = mod
        antenv.axon_hooks = mod
        return True
    except Exception as e:  # tracing is best-effort
        print(f"tracing unavailable: {e}")
        return False


def kernel(x, weight, bias, mask):
    x = np.asarray(x, dtype=np.float32)
    weight = np.asarray(weight, dtype=np.float32)
    bias = np.asarray(bias, dtype=np.float32)
    mask = np.asarray(mask, dtype=np.int32)

    nc = _get_program()

    in_maps = []
    for core in range(8):
        r, c = core // C_SHARDS, core % C_SHARDS
        in_maps.append({
            "x": np.ascontiguousarray(x[r * T_C:(r + 1) * T_C]),
            "w": np.ascontiguousarray(weight[c * O_C:(c + 1) * O_C]),
            "m": np.ascontiguousarray(mask[c * O_C:(c + 1) * O_C]),
            "b": np.ascontiguousarray(bias[c * O_C:(c + 1) * O_C]),
        })

    trace = os.environ.get("KERNEL_TRACE", "1") == "1"
    if trace:
        trace = _enable_tracing()
    res = None
    if trace:
        tmpdir = os.environ.get("KERNEL_TRACE_DIR")
        if tmpdir:
            os.makedirs(tmpdir, exist_ok=True)
        try:
            res = run_bass_kernel_spmd(nc, in_maps, core_ids=list(range(8)),
                                       trace=True, tmpdir=tmpdir)
        except Exception as e:
            print(f"traced run failed ({e!r}); rerunning untraced")
            res = None
    if res is None:
        res = run_bass_kernel_spmd(nc, in_maps, core_ids=list(range(8)))
    if res.exec_time_ns is not None:
        print(f"HW exec time: {res.exec_time_ns} ns")

    out = np.empty((TOKENS, OUTDIM), dtype=np.float32)
    for core in range(8):
        r, c = core // C_SHARDS, core % C_SHARDS
        out[r * T_C:(r + 1) * T_C, c * O_C:(c + 1) * O_C] = \
            np.ascontiguousarray(res.results[core]["yt"].T)
    return out


def _sim_test(t_c=512, o_c=256, k=1024):
    """CoreSim numerics check at reduced size."""
    from concourse.bass_interp import CoreSim
    rng = np.random.default_rng(0)
    xv = rng.standard_normal((t_c, k), dtype=np.float32)
    wv = rng.standard_normal((o_c, k), dtype=np.float32) * 0.03
    mv = rng.integers(0, 2, size=(o_c, k)).astype(np.int32)
    bv = rng.standard_normal(o_c).astype(np.float32)

    nc = build_program(t_c=t_c, o_c=o_c, k=k, hw=False)
    sim = CoreSim(nc)
    sim.tensor("x")[:] = xv
    sim.tensor("w")[:] = wv
    sim.tensor("m")[:] = mv
    sim.tensor("b")[:] = bv
    sim.simulate(check_with_hw=False)
    got = np.array(sim.tensor("yt")).T  # [t_c, o_c]

    wm = wv * mv
    ref = xv @ wm.T + bv
    num = np.linalg.norm((got - ref).astype(np.float64))
    den = np.linalg.norm(ref.astype(np.float64)) + 1e-30
    print(f"sim rel err: {num / den:.6g}  (max abs {np.abs(got - ref).max():.4g})")
    assert num / den < 2e-2, "sim numerics check FAILED"
    print("SIM OK")


if __name__ == "__main__":
    _sim_test()


# revision 17
# speedup vs baseline: 1.3788x; 1.0205x over previous
"""ExpanderLinear on 8 TRN2 NeuronCores — v2: PE does ONLY matmuls.

y = x @ (weight * mask)^T + bias
  x      [8192, 4096] f32
  weight [4096, 4096] f32
  mask   [4096, 4096] i32 (0/1)
  bias   [4096]       f32
  y      [8192, 4096] f32

Sharding: 2D 4x2 grid — 4 token shards x 2 outdim shards (t_c=o_c=2048 per
core). Each core computes yt = (w*m) @ x^T + b (the TRANSPOSED output tile,
[o_c, t_c]); the host transposes shards during unshard.

Device pipeline (all operand prep OFF the PE):
  x:  raw f32 quarter-loads (ACT HWDGE ring) -> DVE cast to bf16 panels ->
      HWDGE xbar DMA-transpose into a resident x^T SBUF buffer
      [128, NP, KT, 128] (k on partitions).
  wm: w f32 loads (ACT queue) + mask i32->bf16 cast loads (SWDGE); DVE
      multiply -> bf16; xbar DMA-transpose into wm^T slivers [128, KT, 128].
  PE: pure bf16 matmul stream: stationary = wm^T sliver tile [128k, 128o],
      moving = x^T [128k, 512t], PSUM accumulates f32 over 32 k-tiles.
      No PE transposes -> no HAM-cold restarts, LDWEIGHTS hidden.
  DVE: PSUM->SBUF eviction fused with per-partition bias (tensor_scalar).
  SWDGE: yt stores.
"""
import os
import sys

sys.path.insert(0, "/opt/trn_rl_repo")

import numpy as np  # noqa: E402

import concourse.bass as bass  # noqa: E402,F401
import concourse.mybir as mybir  # noqa: E402
import concourse.tile as tile  # noqa: E402
import concourse.bacc as bacc  # noqa: E402
from concourse.bass_utils import run_bass_kernel_spmd  # noqa: E402
from concourse.bass_interp import get_hw_module  # noqa: E402

TOKENS, INDIM, OUTDIM = 8192, 4096, 4096
R_SHARDS, C_SHARDS = 4, 2
T_C, O_C = TOKENS // R_SHARDS, OUTDIM // C_SHARDS  # 2048, 2048

P = 128      # partitions / k-tile size
TCH = 512    # token chunk (psum free dim)
KPREP = 1024  # wm prep chunk along k


def build_program(t_c=T_C, o_c=O_C, k=INDIM, hw=True):
    KT = k // P           # k-tiles
    NP = t_c // P         # x panels
    NOC = o_c // P        # out tiles
    NTC = t_c // TCH      # token chunks
    NQ = k // KPREP       # wm prep chunks
    PPT = TCH // P        # panels per token chunk
    NJ = KPREP // P       # k-tiles per prep chunk

    nc = bacc.Bacc("TRN2", target_bir_lowering=False, debug=False,
                   num_devices=8)
    x = nc.dram_tensor("x", [t_c, k], mybir.dt.float32, kind="ExternalInput")
    w = nc.dram_tensor("w", [o_c, k], mybir.dt.float32, kind="ExternalInput")
    m = nc.dram_tensor("m", [o_c, k], mybir.dt.int32, kind="ExternalInput")
    b = nc.dram_tensor("b", [o_c], mybir.dt.float32, kind="ExternalInput")
    yt = nc.dram_tensor("yt", [o_c, t_c], mybir.dt.float32,
                        kind="ExternalOutput")

    with tile.TileContext(nc) as tc:
        with (tc.tile_pool(name="xT_pool", bufs=1) as xT_pool,
              tc.tile_pool(name="biasp", bufs=1) as biasp,
              tc.tile_pool(name="xfstage", bufs=4) as xfstage,
              tc.tile_pool(name="xbpan", bufs=2) as xbpan,
              tc.tile_pool(name="wstage", bufs=3) as wstage,
              tc.tile_pool(name="mstage", bufs=3) as mstage,
              tc.tile_pool(name="wmstage", bufs=3) as wmstage,
              tc.tile_pool(name="slivp", bufs=2) as slivp,
              tc.tile_pool(name="outp", bufs=3) as outp,
              tc.tile_pool(name="consts", bufs=1) as consts,
              tc.tile_pool(name="psum", bufs=6, space="PSUM") as psum_pool,
              tc.tile_pool(name="psum_t", bufs=2, space="PSUM") as psum_t):
            from concourse.masks import make_identity
            xT = xT_pool.tile([P, KT, t_c], mybir.dt.bfloat16, name="xT")
            ident = consts.tile([P, P], mybir.dt.bfloat16, name="ident")
            make_identity(nc, ident)
            bias_sb = biasp.tile([P, NOC], mybir.dt.float32, name="bias_sb")

            for oc in range(NOC):
                nc.gpsimd.dma_start(
                    bias_sb[:, oc:oc + 1],
                    b[oc * P:(oc + 1) * P, None])

            TG = min(8, KT)  # k-tiles per transpose-psum group
            xpan = {}

            def x_panel(p):
                xb = xbpan.tile([P, k], mybir.dt.bfloat16, tag="xbpan")
                xpan[p] = xb
                ldq = nc.scalar if p % 2 == 0 else nc.sync
                for q in range(max(1, k // KPREP)):
                    cw = min(KPREP, k)
                    k0 = q * cw
                    xf = xfstage.tile([P, cw], mybir.dt.float32, tag="xf")
                    ldq.dma_start(xf[:, :],
                                  x[p * P:(p + 1) * P, k0:k0 + cw])
                    nc.vector.tensor_copy(xb[:, k0:k0 + cw], xf[:, :])

            def x_tr(p):
                # PE transposes the bf16 panel into resident x^T
                xb = xpan.pop(p)
                for g in range(KT // TG):
                    pt = psum_t.tile([P, TG, P], mybir.dt.bfloat16,
                                     tag="tpsum")
                    for j in range(TG):
                        kt = g * TG + j
                        nc.tensor.transpose(pt[:, j, :],
                                            xb[:, kt * P:(kt + 1) * P],
                                            ident[:, :])
                    nc.vector.tensor_copy(
                        xT[:, g * TG:(g + 1) * TG, p * P:(p + 1) * P],
                        pt[:, :, :])

            slivers = {}

            def wm_prep(oc):
                sl = slivp.tile([P, KT, P], mybir.dt.bfloat16, tag="sliv")
                slivers[oc] = sl
                ro = oc * P
                for q in range(NQ):
                    ks = slice(q * KPREP, (q + 1) * KPREP)
                    wch = wstage.tile([P, KPREP], mybir.dt.float32, tag="wch")
                    nc.scalar.dma_start(wch[:, :], w[ro:ro + P, ks])
                    mch = mstage.tile([P, KPREP], mybir.dt.bfloat16, tag="mch")
                    nc.gpsimd.dma_start(mch[:, :], m[ro:ro + P, ks])
                    wmb = wmstage.tile([P, KPREP], mybir.dt.bfloat16,
                                       tag="wmb")
                    nc.vector.tensor_mul(wmb[:, :], wch[:, :], mch[:, :])
                    nc.sync.dma_start(sl[:, q * NJ:(q + 1) * NJ, :],
                                      wmb[:, :], transpose=True)

            def evict(oc, tcx, pt):
                ot = outp.tile([P, TCH], mybir.dt.float32, tag="out")
                nc.vector.tensor_scalar_add(ot[:, :], pt[:, :],
                                            bias_sb[:, oc:oc + 1])
                nc.gpsimd.dma_start(
                    yt[oc * P:(oc + 1) * P, tcx * TCH:(tcx + 1) * TCH],
                    ot[:, :])

            def mm_group(pt, sl, tcx, kt):
                nc.tensor.matmul(
                    pt[:, :], sl[:, kt, :],
                    xT[:, kt, tcx * TCH:(tcx + 1) * TCH],
                    start=(kt == 0), stop=(kt == KT - 1))

            # ---- emission ----
            for tcx in range(NTC):
                for p in range(tcx * PPT, min((tcx + 1) * PPT, NP)):
                    x_panel(p)
                if tcx < min(2, NOC):
                    wm_prep(tcx)
            for _oc in range(min(2, NOC)):
                if _oc not in slivers:
                    wm_prep(_oc)

            for oc in range(NOC):
                sl = slivers.pop(oc)
                if oc == 0:
                    # tc-outer: follow x panel arrival order; PE transposes
                    # each tc-chunk's panels right before its matmul group
                    for tcx in range(NTC):
                        for p in range(tcx * PPT,
                                       min((tcx + 1) * PPT, NP)):
                            x_tr(p)
                        pt = psum_pool.tile([P, TCH], mybir.dt.float32,
                                            tag="acc")
                        for kt in range(KT):
                            mm_group(pt, sl, tcx, kt)
                        evict(oc, tcx, pt)
                else:
                    if oc + 1 < NOC:
                        wm_prep(oc + 1)
                    pts = []
                    for _i in range(NTC):
                        pt = psum_pool.tile([P, TCH], mybir.dt.float32,
                                            tag="acc", name=f"acc{_i}")
                        pts.append(pt)
                    for kt in range(KT):
                        for tcx in range(NTC):
                            mm_group(pts[tcx], sl, tcx, kt)
                    for tcx in range(NTC):
                        evict(oc, tcx, pts[tcx])

    nc.compile()
    if hw:
        nc.m = get_hw_module(nc.m)
    return nc


_PROGRAM = None


def _get_program():
    global _PROGRAM
    if _PROGRAM is None:
        _PROGRAM = build_program()
    return _PROGRAM


def _enable_tracing():
    """Install the axon NTFF profile hook if the image's antenv lacks it."""
    try:
        import contextlib
        import ctypes
        import types

        import concourse.bass_utils as bu
        bu.upload_artifacts = lambda tmpdir: ""  # no S3 in this container

        try:
            from antenv.axon_hooks import get_axon_ntff_profile_hook
            if get_axon_ntff_profile_hook() is not None:
                return True
        except ImportError:
            pass

        so_path = "/opt/axon/libaxon_pjrt.so"
        if not os.path.exists(so_path):
            return False
        lib = ctypes.CDLL(so_path)
        if not hasattr(lib, "axon_start_nrt_profile"):
            return False
        lib.axon_start_nrt_profile.argtypes = [
            ctypes.POINTER(ctypes.c_int64), ctypes.c_size_t]
        lib.axon_start_nrt_profile.restype = ctypes.c_int64
        lib.axon_stop_nrt_profile.argtypes = [ctypes.c_char_p]
        lib.axon_stop_nrt_profile.restype = ctypes.c_int64

        @contextlib.contextmanager
        def _hook(output_dir, device_ids):
            import jax
            jax.devices()
            if device_ids:
                ids = (ctypes.c_int64 * len(device_ids))(*device_ids)
                rc = lib.axon_start_nrt_profile(ids, len(device_ids))
            else:
                rc = lib.axon_start_nrt_profile(None, 0)
            if rc != 0:
                raise RuntimeError(f"axon_start_nrt_profile rc={rc}")
            try:
                yield
            finally:
                n = lib.axon_stop_nrt_profile(str(output_dir).encode())
                if n <= 0:
                    print(f"ntff profile: rc={n} (no files) -> {output_dir}")

        mod = types.ModuleType("antenv.axon_hooks")
        _state = {"hook": _hook}
        mod.set_axon_ntff_profile_hook = lambda h: _state.update(hook=h)
        mod.get_axon_ntff_profile_hook = lambda: _state["hook"]
        import antenv
        sys.modules["antenv.axon_hooks"] = mod
        antenv.axon_hooks = mod
        return True
    except Exception as e:  # tracing is best-effort
        print(f"tracing unavailable: {e}")
        return False


def kernel(x, weight, bias, mask):
    x = np.asarray(x, dtype=np.float32)
    weight = np.asarray(weight, dtype=np.float32)
    bias = np.asarray(bias, dtype=np.float32)
    mask = np.asarray(mask, dtype=np.int32)

    nc = _get_program()

    in_maps = []
    for core in range(8):
        r, c = core // C_SHARDS, core % C_SHARDS
        in_maps.append({
            "x": np.ascontiguousarray(x[r * T_C:(r + 1) * T_C]),
            "w": np.ascontiguousarray(weight[c * O_C:(c + 1) * O_C]),
            "m": np.ascontiguousarray(mask[c * O_C:(c + 1) * O_C]),
            "b": np.ascontiguousarray(bias[c * O_C:(c + 1) * O_C]),
        })

    trace = os.environ.get("KERNEL_TRACE", "1") == "1"
    if trace:
        trace = _enable_tracing()
    res = None
    if trace:
        tmpdir = os.environ.get("KERNEL_TRACE_DIR")
        if tmpdir:
            os.makedirs(tmpdir, exist_ok=True)
        try:
            res = run_bass_kernel_spmd(nc, in_maps, core_ids=list(range(8)),
                                       trace=True, tmpdir=tmpdir)
        except Exception as e:
            print(f"traced run failed ({e!r}); rerunning untraced")
            res = None
    if res is None:
        res = run_bass_kernel_spmd(nc, in_maps, core_ids=list(range(8)))
    if res.exec_time_ns is not None:
        print(f"HW exec time: {res.exec_time_ns} ns")

    out = np.empty((TOKENS, OUTDIM), dtype=np.float32)
    for core in range(8):
        r, c = core // C_SHARDS, core % C_SHARDS
        out[r * T_C:(r + 1) * T_C, c * O_C:(c + 1) * O_C] = \
            np.ascontiguousarray(res.results[core]["yt"].T)
    return out


def _sim_test(t_c=512, o_c=256, k=1024):
    """CoreSim numerics check at reduced size."""
    from concourse.bass_interp import CoreSim
    rng = np.random.default_rng(0)
    xv = rng.standard_normal((t_c, k), dtype=np.float32)
    wv = rng.standard_normal((o_c, k), dtype=np.float32) * 0.03
    mv = rng.integers(0, 2, size=(o_c, k)).astype(np.int32)
    bv = rng.standard_normal(o_c).astype(np.float32)

    nc = build_program(t_c=t_c, o_c=o_c, k=k, hw=False)
    sim = CoreSim(nc)
    sim.tensor("x")[:] = xv
    sim.tensor("w")[:] = wv
    sim.tensor("m")[:] = mv
    sim.tensor("b")[:] = bv
    sim.simulate(check_with_hw=False)
    got = np.array(sim.tensor("yt")).T  # [t_c, o_c]

    wm = wv * mv
    ref = xv @ wm.T + bv
    num = np.linalg.norm((got - ref).astype(np.float64))
    den = np.linalg.norm(ref.astype(np.float64)) + 1e-30
    print(f"sim rel err: {num / den:.6g}  (max abs {np.abs(got - ref).max():.4g})")
    assert num / den < 2e-2, "sim numerics check FAILED"
    print("SIM OK")


if __name__ == "__main__":
    _sim_test()


# revision 18
# speedup vs baseline: 1.4344x; 1.0403x over previous
"""ExpanderLinear on 8 TRN2 NeuronCores — v2: PE does ONLY matmuls.

y = x @ (weight * mask)^T + bias
  x      [8192, 4096] f32
  weight [4096, 4096] f32
  mask   [4096, 4096] i32 (0/1)
  bias   [4096]       f32
  y      [8192, 4096] f32

Sharding: 2D 4x2 grid — 4 token shards x 2 outdim shards (t_c=o_c=2048 per
core). Each core computes yt = (w*m) @ x^T + b (the TRANSPOSED output tile,
[o_c, t_c]); the host transposes shards during unshard.

Device pipeline (all operand prep OFF the PE):
  x:  raw f32 quarter-loads (ACT HWDGE ring) -> DVE cast to bf16 panels ->
      HWDGE xbar DMA-transpose into a resident x^T SBUF buffer
      [128, NP, KT, 128] (k on partitions).
  wm: w f32 loads (ACT queue) + mask i32->bf16 cast loads (SWDGE); DVE
      multiply -> bf16; xbar DMA-transpose into wm^T slivers [128, KT, 128].
  PE: pure bf16 matmul stream: stationary = wm^T sliver tile [128k, 128o],
      moving = x^T [128k, 512t], PSUM accumulates f32 over 32 k-tiles.
      No PE transposes -> no HAM-cold restarts, LDWEIGHTS hidden.
  DVE: PSUM->SBUF eviction fused with per-partition bias (tensor_scalar).
  SWDGE: yt stores.
"""
import os
import sys

sys.path.insert(0, "/opt/trn_rl_repo")

import numpy as np  # noqa: E402

import concourse.bass as bass  # noqa: E402,F401
import concourse.mybir as mybir  # noqa: E402
import concourse.tile as tile  # noqa: E402
import concourse.bacc as bacc  # noqa: E402
from concourse.bass_utils import run_bass_kernel_spmd  # noqa: E402
from concourse.bass_interp import get_hw_module  # noqa: E402

TOKENS, INDIM, OUTDIM = 8192, 4096, 4096
R_SHARDS, C_SHARDS = 4, 2
T_C, O_C = TOKENS // R_SHARDS, OUTDIM // C_SHARDS  # 2048, 2048

P = 128      # partitions / k-tile size
TCH = 512    # token chunk (psum free dim)
KPREP = 1024  # wm prep chunk along k


def build_program(t_c=T_C, o_c=O_C, k=INDIM, hw=True):
    KT = k // P           # k-tiles
    NP = t_c // P         # x panels
    NOC = o_c // P        # out tiles
    NTC = t_c // TCH      # token chunks
    NQ = k // KPREP       # wm prep chunks
    PPT = TCH // P        # panels per token chunk
    NJ = KPREP // P       # k-tiles per prep chunk

    nc = bacc.Bacc("TRN2", target_bir_lowering=False, debug=False,
                   num_devices=8)
    x = nc.dram_tensor("x", [t_c, k], mybir.dt.float32, kind="ExternalInput")
    w = nc.dram_tensor("w", [o_c, k], mybir.dt.float32, kind="ExternalInput")
    m = nc.dram_tensor("m", [o_c, k], mybir.dt.int32, kind="ExternalInput")
    b = nc.dram_tensor("b", [o_c], mybir.dt.float32, kind="ExternalInput")
    yt = nc.dram_tensor("yt", [o_c, t_c], mybir.dt.float32,
                        kind="ExternalOutput")

    with tile.TileContext(nc) as tc:
        with (tc.tile_pool(name="xT_pool", bufs=1) as xT_pool,
              tc.tile_pool(name="biasp", bufs=1) as biasp,
              tc.tile_pool(name="xfstage", bufs=4) as xfstage,
              tc.tile_pool(name="xbpan", bufs=2) as xbpan,
              tc.tile_pool(name="wstage", bufs=3) as wstage,
              tc.tile_pool(name="mstage", bufs=3) as mstage,
              tc.tile_pool(name="wmstage", bufs=3) as wmstage,
              tc.tile_pool(name="slivp", bufs=2) as slivp,
              tc.tile_pool(name="outp", bufs=3) as outp,
              tc.tile_pool(name="consts", bufs=1) as consts,
              tc.tile_pool(name="psum", bufs=6, space="PSUM") as psum_pool,
              tc.tile_pool(name="psum_t", bufs=2, space="PSUM") as psum_t):
            from concourse.masks import make_identity
            xT = xT_pool.tile([P, KT, t_c], mybir.dt.bfloat16, name="xT")
            ident = consts.tile([P, P], mybir.dt.bfloat16, name="ident")
            make_identity(nc, ident)
            bias_sb = biasp.tile([P, NOC], mybir.dt.float32, name="bias_sb")

            for oc in range(NOC):
                nc.gpsimd.dma_start(
                    bias_sb[:, oc:oc + 1],
                    b[oc * P:(oc + 1) * P, None])

            TG = min(8, KT)  # k-tiles per transpose-psum group
            xpan = {}

            def x_panel(p):
                xb = xbpan.tile([P, k], mybir.dt.bfloat16, tag="xbpan")
                xpan[p] = xb
                ldq = nc.scalar if p % 2 == 0 else nc.sync
                for q in range(max(1, k // KPREP)):
                    cw = min(KPREP, k)
                    k0 = q * cw
                    xf = xfstage.tile([P, cw], mybir.dt.float32, tag="xf")
                    ldq.dma_start(xf[:, :],
                                  x[p * P:(p + 1) * P, k0:k0 + cw])
                    nc.vector.tensor_copy(xb[:, k0:k0 + cw], xf[:, :])

            def x_tr(p):
                # PE transposes the bf16 panel into resident x^T
                xb = xpan.pop(p)
                for g in range(KT // TG):
                    pt = psum_t.tile([P, TG, P], mybir.dt.bfloat16,
                                     tag="tpsum")
                    for j in range(TG):
                        kt = g * TG + j
                        nc.tensor.transpose(pt[:, j, :],
                                            xb[:, kt * P:(kt + 1) * P],
                                            ident[:, :])
                    nc.vector.tensor_copy(
                        xT[:, g * TG:(g + 1) * TG, p * P:(p + 1) * P],
                        pt[:, :, :])

            slivers = {}

            def wm_prep(oc):
                sl = slivp.tile([P, KT, P], mybir.dt.bfloat16, tag="sliv")
                slivers[oc] = sl
                ro = oc * P
                for q in range(NQ):
                    ks = slice(q * KPREP, (q + 1) * KPREP)
                    wch = wstage.tile([P, KPREP], mybir.dt.float32, tag="wch")
                    nc.scalar.dma_start(wch[:, :], w[ro:ro + P, ks])
                    mch = mstage.tile([P, KPREP], mybir.dt.bfloat16, tag="mch")
                    nc.gpsimd.dma_start(mch[:, :], m[ro:ro + P, ks])
                    wmb = wmstage.tile([P, KPREP], mybir.dt.bfloat16,
                                       tag="wmb")
                    nc.vector.tensor_mul(wmb[:, :], wch[:, :], mch[:, :])
                    nc.sync.dma_start(sl[:, q * NJ:(q + 1) * NJ, :],
                                      wmb[:, :], transpose=True)

            def evict(oc, tcx, pt):
                ot = outp.tile([P, TCH], mybir.dt.float32, tag="out")
                nc.vector.tensor_scalar_add(ot[:, :], pt[:, :],
                                            bias_sb[:, oc:oc + 1])
                nc.gpsimd.dma_start(
                    yt[oc * P:(oc + 1) * P, tcx * TCH:(tcx + 1) * TCH],
                    ot[:, :])

            def mm_group(pt, sl, tcx, kt):
                nc.tensor.matmul(
                    pt[:, :], sl[:, kt, :],
                    xT[:, kt, tcx * TCH:(tcx + 1) * TCH],
                    start=(kt == 0), stop=(kt == KT - 1))

            # ---- emission ----
            for tcx in range(NTC):
                for p in range(tcx * PPT, min((tcx + 1) * PPT, NP)):
                    x_panel(p)
                if tcx < min(2, NOC):
                    wm_prep(tcx)
            for _oc in range(min(2, NOC)):
                if _oc not in slivers:
                    wm_prep(_oc)

            for oc in range(NOC):
                sl = slivers.pop(oc)
                if oc == 0:
                    # tc-outer: follow x panel arrival order. PE transposes
                    # tc0 up front; later chunks' transposes interleave late
                    # into the previous chunk's k-loop (casts land by then)
                    for p in range(0, min(PPT, NP)):
                        x_tr(p)
                    for tcx in range(NTC):
                        nxt = list(range((tcx + 1) * PPT,
                                         min((tcx + 2) * PPT, NP)))
                        pt = psum_pool.tile([P, TCH], mybir.dt.float32,
                                            tag="acc")
                        for kt in range(KT):
                            if nxt and kt >= KT // 2 and (KT - kt) % 5 == 0:
                                x_tr(nxt.pop(0))
                            mm_group(pt, sl, tcx, kt)
                        for p in nxt:
                            x_tr(p)
                        evict(oc, tcx, pt)
                else:
                    if oc + 1 < NOC:
                        wm_prep(oc + 1)
                    pts = []
                    for _i in range(NTC):
                        pt = psum_pool.tile([P, TCH], mybir.dt.float32,
                                            tag="acc", name=f"acc{_i}")
                        pts.append(pt)
                    for kt in range(KT):
                        for tcx in range(NTC):
                            mm_group(pts[tcx], sl, tcx, kt)
                    for tcx in range(NTC):
                        evict(oc, tcx, pts[tcx])

    nc.compile()
    if hw:
        nc.m = get_hw_module(nc.m)
    return nc


_PROGRAM = None


def _get_program():
    global _PROGRAM
    if _PROGRAM is None:
        _PROGRAM = build_program()
    return _PROGRAM


def _enable_tracing():
    """Install the axon NTFF profile hook if the image's antenv lacks it."""
    try:
        import contextlib
        import ctypes
        import types

        import concourse.bass_utils as bu
        bu.upload_artifacts = lambda tmpdir: ""  # no S3 in this container

        try:
            from antenv.axon_hooks import get_axon_ntff_profile_hook
            if get_axon_ntff_profile_hook() is not None:
                return True
        except ImportError:
            pass

        so_path = "/opt/axon/libaxon_pjrt.so"
        if not os.path.exists(so_path):
            return False
        lib = ctypes.CDLL(so_path)
        if not hasattr(lib, "axon_start_nrt_profile"):
            return False
        lib.axon_start_nrt_profile.argtypes = [
            ctypes.POINTER(ctypes.c_int64), ctypes.c_size_t]
        lib.axon_start_nrt_profile.restype = ctypes.c_int64
        lib.axon_stop_nrt_profile.argtypes = [ctypes.c_char_p]
        lib.axon_stop_nrt_profile.restype = ctypes.c_int64

        @contextlib.contextmanager
        def _hook(output_dir, device_ids):
            import jax
            jax.devices()
            if device_ids:
                ids = (ctypes.c_int64 * len(device_ids))(*device_ids)
                rc = lib.axon_start_nrt_profile(ids, len(device_ids))
            else:
                rc = lib.axon_start_nrt_profile(None, 0)
            if rc != 0:
                raise RuntimeError(f"axon_start_nrt_profile rc={rc}")
            try:
                yield
            finally:
                n = lib.axon_stop_nrt_profile(str(output_dir).encode())
                if n <= 0:
                    print(f"ntff profile: rc={n} (no files) -> {output_dir}")

        mod = types.ModuleType("antenv.axon_hooks")
        _state = {"hook": _hook}
        mod.set_axon_ntff_profile_hook = lambda h: _state.update(hook=h)
        mod.get_axon_ntff_profile_hook = lambda: _state["hook"]
        import antenv
        sys.modules["antenv.axon_hooks"] = mod
        antenv.axon_hooks = mod
        return True
    except Exception as e:  # tracing is best-effort
        print(f"tracing unavailable: {e}")
        return False


def kernel(x, weight, bias, mask):
    x = np.asarray(x, dtype=np.float32)
    weight = np.asarray(weight, dtype=np.float32)
    bias = np.asarray(bias, dtype=np.float32)
    mask = np.asarray(mask, dtype=np.int32)

    nc = _get_program()

    in_maps = []
    for core in range(8):
        r, c = core // C_SHARDS, core % C_SHARDS
        in_maps.append({
            "x": np.ascontiguousarray(x[r * T_C:(r + 1) * T_C]),
            "w": np.ascontiguousarray(weight[c * O_C:(c + 1) * O_C]),
            "m": np.ascontiguousarray(mask[c * O_C:(c + 1) * O_C]),
            "b": np.ascontiguousarray(bias[c * O_C:(c + 1) * O_C]),
        })

    trace = os.environ.get("KERNEL_TRACE", "1") == "1"
    if trace:
        trace = _enable_tracing()
    res = None
    if trace:
        tmpdir = os.environ.get("KERNEL_TRACE_DIR")
        if tmpdir:
            os.makedirs(tmpdir, exist_ok=True)
        try:
            res = run_bass_kernel_spmd(nc, in_maps, core_ids=list(range(8)),
                                       trace=True, tmpdir=tmpdir)
        except Exception as e:
            print(f"traced run failed ({e!r}); rerunning untraced")
            res = None
    if res is None:
        res = run_bass_kernel_spmd(nc, in_maps, core_ids=list(range(8)))
    if res.exec_time_ns is not None:
        print(f"HW exec time: {res.exec_time_ns} ns")

    out = np.empty((TOKENS, OUTDIM), dtype=np.float32)
    for core in range(8):
        r, c = core // C_SHARDS, core % C_SHARDS
        out[r * T_C:(r + 1) * T_C, c * O_C:(c + 1) * O_C] = \
            np.ascontiguousarray(res.results[core]["yt"].T)
    return out


def _sim_test(t_c=512, o_c=256, k=1024):
    """CoreSim numerics check at reduced size."""
    from concourse.bass_interp import CoreSim
    rng = np.random.default_rng(0)
    xv = rng.standard_normal((t_c, k), dtype=np.float32)
    wv = rng.standard_normal((o_c, k), dtype=np.float32) * 0.03
    mv = rng.integers(0, 2, size=(o_c, k)).astype(np.int32)
    bv = rng.standard_normal(o_c).astype(np.float32)

    nc = build_program(t_c=t_c, o_c=o_c, k=k, hw=False)
    sim = CoreSim(nc)
    sim.tensor("x")[:] = xv
    sim.tensor("w")[:] = wv
    sim.tensor("m")[:] = mv
    sim.tensor("b")[:] = bv
    sim.simulate(check_with_hw=False)
    got = np.array(sim.tensor("yt")).T  # [t_c, o_c]

    wm = wv * mv
    ref = xv @ wm.T + bv
    num = np.linalg.norm((got - ref).astype(np.float64))
    den = np.linalg.norm(ref.astype(np.float64)) + 1e-30
    print(f"sim rel err: {num / den:.6g}  (max abs {np.abs(got - ref).max():.4g})")
    assert num / den < 2e-2, "sim numerics check FAILED"
    print("SIM OK")


if __name__ == "__main__":
    _sim_test()


# revision 19
# speedup vs baseline: 1.4440x; 1.0067x over previous
"""ExpanderLinear on 8 TRN2 NeuronCores — v2: PE does ONLY matmuls.

y = x @ (weight * mask)^T + bias
  x      [8192, 4096] f32
  weight [4096, 4096] f32
  mask   [4096, 4096] i32 (0/1)
  bias   [4096]       f32
  y      [8192, 4096] f32

Sharding: 2D 4x2 grid — 4 token shards x 2 outdim shards (t_c=o_c=2048 per
core). Each core computes yt = (w*m) @ x^T + b (the TRANSPOSED output tile,
[o_c, t_c]); the host transposes shards during unshard.

Device pipeline (all operand prep OFF the PE):
  x:  raw f32 quarter-loads (ACT HWDGE ring) -> DVE cast to bf16 panels ->
      HWDGE xbar DMA-transpose into a resident x^T SBUF buffer
      [128, NP, KT, 128] (k on partitions).
  wm: w f32 loads (ACT queue) + mask i32->bf16 cast loads (SWDGE); DVE
      multiply -> bf16; xbar DMA-transpose into wm^T slivers [128, KT, 128].
  PE: pure bf16 matmul stream: stationary = wm^T sliver tile [128k, 128o],
      moving = x^T [128k, 512t], PSUM accumulates f32 over 32 k-tiles.
      No PE transposes -> no HAM-cold restarts, LDWEIGHTS hidden.
  DVE: PSUM->SBUF eviction fused with per-partition bias (tensor_scalar).
  SWDGE: yt stores.
"""
import os
import sys

sys.path.insert(0, "/opt/trn_rl_repo")

import numpy as np  # noqa: E402

import concourse.bass as bass  # noqa: E402,F401
import concourse.mybir as mybir  # noqa: E402
import concourse.tile as tile  # noqa: E402
import concourse.bacc as bacc  # noqa: E402
from concourse.bass_utils import run_bass_kernel_spmd  # noqa: E402
from concourse.bass_interp import get_hw_module  # noqa: E402

TOKENS, INDIM, OUTDIM = 8192, 4096, 4096
R_SHARDS, C_SHARDS = 4, 2
T_C, O_C = TOKENS // R_SHARDS, OUTDIM // C_SHARDS  # 2048, 2048

P = 128      # partitions / k-tile size
TCH = 512    # token chunk (psum free dim)
KPREP = 1024  # wm prep chunk along k


def build_program(t_c=T_C, o_c=O_C, k=INDIM, hw=True):
    KT = k // P           # k-tiles
    NP = t_c // P         # x panels
    NOC = o_c // P        # out tiles
    NTC = t_c // TCH      # token chunks
    NQ = k // KPREP       # wm prep chunks
    PPT = TCH // P        # panels per token chunk
    NJ = KPREP // P       # k-tiles per prep chunk

    nc = bacc.Bacc("TRN2", target_bir_lowering=False, debug=False,
                   num_devices=8)
    x = nc.dram_tensor("x", [t_c, k], mybir.dt.float32, kind="ExternalInput")
    w = nc.dram_tensor("w", [o_c, k], mybir.dt.float32, kind="ExternalInput")
    m = nc.dram_tensor("m", [o_c, k], mybir.dt.int32, kind="ExternalInput")
    b = nc.dram_tensor("b", [o_c], mybir.dt.float32, kind="ExternalInput")
    yt = nc.dram_tensor("yt", [o_c, t_c], mybir.dt.float32,
                        kind="ExternalOutput")

    with tile.TileContext(nc) as tc:
        with (tc.tile_pool(name="xT_pool", bufs=1) as xT_pool,
              tc.tile_pool(name="biasp", bufs=1) as biasp,
              tc.tile_pool(name="xfstage", bufs=4) as xfstage,
              tc.tile_pool(name="xbpan", bufs=2) as xbpan,
              tc.tile_pool(name="wstage", bufs=3) as wstage,
              tc.tile_pool(name="mstage", bufs=3) as mstage,
              tc.tile_pool(name="wmstage", bufs=3) as wmstage,
              tc.tile_pool(name="slivp", bufs=2) as slivp,
              tc.tile_pool(name="outp", bufs=3) as outp,
              tc.tile_pool(name="consts", bufs=1) as consts,
              tc.tile_pool(name="psum", bufs=6, space="PSUM") as psum_pool,
              tc.tile_pool(name="psum_t", bufs=2, space="PSUM") as psum_t):
            from concourse.masks import make_identity
            xT = xT_pool.tile([P, KT, t_c], mybir.dt.bfloat16, name="xT")
            ident = consts.tile([P, P], mybir.dt.bfloat16, name="ident")
            make_identity(nc, ident)
            bias_sb = biasp.tile([P, NOC], mybir.dt.float32, name="bias_sb")

            for oc in range(NOC):
                nc.gpsimd.dma_start(
                    bias_sb[:, oc:oc + 1],
                    b[oc * P:(oc + 1) * P, None])

            TG = min(8, KT)  # k-tiles per transpose-psum group
            xpan = {}

            def x_panel(p):
                xb = xbpan.tile([P, k], mybir.dt.bfloat16, tag="xbpan")
                xpan[p] = xb
                ldq = nc.scalar if p % 2 == 0 else nc.sync
                for q in range(max(1, k // KPREP)):
                    cw = min(KPREP, k)
                    k0 = q * cw
                    xf = xfstage.tile([P, cw], mybir.dt.float32, tag="xf")
                    ldq.dma_start(xf[:, :],
                                  x[p * P:(p + 1) * P, k0:k0 + cw])
                    nc.vector.tensor_copy(xb[:, k0:k0 + cw], xf[:, :])

            def x_tr(p):
                # PE transposes the bf16 panel into resident x^T
                xb = xpan.pop(p)
                for g in range(KT // TG):
                    pt = psum_t.tile([P, TG, P], mybir.dt.bfloat16,
                                     tag="tpsum")
                    for j in range(TG):
                        kt = g * TG + j
                        nc.tensor.transpose(pt[:, j, :],
                                            xb[:, kt * P:(kt + 1) * P],
                                            ident[:, :])
                    nc.vector.tensor_copy(
                        xT[:, g * TG:(g + 1) * TG, p * P:(p + 1) * P],
                        pt[:, :, :])

            slivers = {}

            def wm_prep(oc):
                sl = slivp.tile([P, KT, P], mybir.dt.bfloat16, tag="sliv")
                slivers[oc] = sl
                ro = oc * P
                for q in range(NQ):
                    ks = slice(q * KPREP, (q + 1) * KPREP)
                    wch = wstage.tile([P, KPREP], mybir.dt.float32, tag="wch")
                    nc.scalar.dma_start(wch[:, :], w[ro:ro + P, ks])
                    mch = mstage.tile([P, KPREP], mybir.dt.bfloat16, tag="mch")
                    nc.gpsimd.dma_start(mch[:, :], m[ro:ro + P, ks])
                    wmb = wmstage.tile([P, KPREP], mybir.dt.bfloat16,
                                       tag="wmb")
                    nc.vector.tensor_mul(wmb[:, :], wch[:, :], mch[:, :])
                    nc.sync.dma_start(sl[:, q * NJ:(q + 1) * NJ, :],
                                      wmb[:, :], transpose=True)

            def evict(oc, tcx, pt):
                ot = outp.tile([P, TCH], mybir.dt.float32, tag="out")
                nc.vector.tensor_scalar_add(ot[:, :], pt[:, :],
                                            bias_sb[:, oc:oc + 1])
                nc.sync.dma_start(
                    yt[oc * P:(oc + 1) * P, tcx * TCH:(tcx + 1) * TCH],
                    ot[:, :])

            def mm_group(pt, sl, tcx, kt):
                nc.tensor.matmul(
                    pt[:, :], sl[:, kt, :],
                    xT[:, kt, tcx * TCH:(tcx + 1) * TCH],
                    start=(kt == 0), stop=(kt == KT - 1))

            # ---- emission ----
            for tcx in range(NTC):
                for p in range(tcx * PPT, min((tcx + 1) * PPT, NP)):
                    x_panel(p)
                if tcx < min(2, NOC):
                    wm_prep(tcx)
            for _oc in range(min(2, NOC)):
                if _oc not in slivers:
                    wm_prep(_oc)

            for oc in range(NOC):
                sl = slivers.pop(oc)
                if oc == 0:
                    # tc-outer: follow x panel arrival order. PE transposes
                    # tc0 up front; later chunks' transposes interleave late
                    # into the previous chunk's k-loop (casts land by then)
                    for p in range(0, min(PPT, NP)):
                        x_tr(p)
                    for tcx in range(NTC):
                        nxt = list(range((tcx + 1) * PPT,
                                         min((tcx + 2) * PPT, NP)))
                        pt = psum_pool.tile([P, TCH], mybir.dt.float32,
                                            tag="acc")
                        for kt in range(KT):
                            if nxt and kt >= KT // 2 and (KT - kt) % 5 == 0:
                                x_tr(nxt.pop(0))
                            mm_group(pt, sl, tcx, kt)
                        for p in nxt:
                            x_tr(p)
                        evict(oc, tcx, pt)
                else:
                    if oc + 1 < NOC:
                        wm_prep(oc + 1)
                    pts = []
                    for _i in range(NTC):
                        pt = psum_pool.tile([P, TCH], mybir.dt.float32,
                                            tag="acc", name=f"acc{_i}")
                        pts.append(pt)
                    for kt in range(KT):
                        for tcx in range(NTC):
                            mm_group(pts[tcx], sl, tcx, kt)
                    for tcx in range(NTC):
                        evict(oc, tcx, pts[tcx])

    nc.compile()
    if hw:
        nc.m = get_hw_module(nc.m)
    return nc


_PROGRAM = None


def _get_program():
    global _PROGRAM
    if _PROGRAM is None:
        _PROGRAM = build_program()
    return _PROGRAM


def _enable_tracing():
    """Install the axon NTFF profile hook if the image's antenv lacks it."""
    try:
        import contextlib
        import ctypes
        import types

        import concourse.bass_utils as bu
        bu.upload_artifacts = lambda tmpdir: ""  # no S3 in this container

        try:
            from antenv.axon_hooks import get_axon_ntff_profile_hook
            if get_axon_ntff_profile_hook() is not None:
                return True
        except ImportError:
            pass

        so_path = "/opt/axon/libaxon_pjrt.so"
        if not os.path.exists(so_path):
            return False
        lib = ctypes.CDLL(so_path)
        if not hasattr(lib, "axon_start_nrt_profile"):
            return False
        lib.axon_start_nrt_profile.argtypes = [
            ctypes.POINTER(ctypes.c_int64), ctypes.c_size_t]
        lib.axon_start_nrt_profile.restype = ctypes.c_int64
        lib.axon_stop_nrt_profile.argtypes = [ctypes.c_char_p]
        lib.axon_stop_nrt_profile.restype = ctypes.c_int64

        @contextlib.contextmanager
        def _hook(output_dir, device_ids):
            import jax
            jax.devices()
            if device_ids:
                ids = (ctypes.c_int64 * len(device_ids))(*device_ids)
                rc = lib.axon_start_nrt_profile(ids, len(device_ids))
            else:
                rc = lib.axon_start_nrt_profile(None, 0)
            if rc != 0:
                raise RuntimeError(f"axon_start_nrt_profile rc={rc}")
            try:
                yield
            finally:
                n = lib.axon_stop_nrt_profile(str(output_dir).encode())
                if n <= 0:
                    print(f"ntff profile: rc={n} (no files) -> {output_dir}")

        mod = types.ModuleType("antenv.axon_hooks")
        _state = {"hook": _hook}
        mod.set_axon_ntff_profile_hook = lambda h: _state.update(hook=h)
        mod.get_axon_ntff_profile_hook = lambda: _state["hook"]
        import antenv
        sys.modules["antenv.axon_hooks"] = mod
        antenv.axon_hooks = mod
        return True
    except Exception as e:  # tracing is best-effort
        print(f"tracing unavailable: {e}")
        return False


def kernel(x, weight, bias, mask):
    x = np.asarray(x, dtype=np.float32)
    weight = np.asarray(weight, dtype=np.float32)
    bias = np.asarray(bias, dtype=np.float32)
    mask = np.asarray(mask, dtype=np.int32)

    nc = _get_program()

    in_maps = []
    for core in range(8):
        r, c = core // C_SHARDS, core % C_SHARDS
        in_maps.append({
            "x": np.ascontiguousarray(x[r * T_C:(r + 1) * T_C]),
            "w": np.ascontiguousarray(weight[c * O_C:(c + 1) * O_C]),
            "m": np.ascontiguousarray(mask[c * O_C:(c + 1) * O_C]),
            "b": np.ascontiguousarray(bias[c * O_C:(c + 1) * O_C]),
        })

    trace = os.environ.get("KERNEL_TRACE", "1") == "1"
    if trace:
        trace = _enable_tracing()
    res = None
    if trace:
        tmpdir = os.environ.get("KERNEL_TRACE_DIR")
        if tmpdir:
            os.makedirs(tmpdir, exist_ok=True)
        try:
            res = run_bass_kernel_spmd(nc, in_maps, core_ids=list(range(8)),
                                       trace=True, tmpdir=tmpdir)
        except Exception as e:
            print(f"traced run failed ({e!r}); rerunning untraced")
            res = None
    if res is None:
        res = run_bass_kernel_spmd(nc, in_maps, core_ids=list(range(8)))
    if res.exec_time_ns is not None:
        print(f"HW exec time: {res.exec_time_ns} ns")

    out = np.empty((TOKENS, OUTDIM), dtype=np.float32)
    for core in range(8):
        r, c = core // C_SHARDS, core % C_SHARDS
        out[r * T_C:(r + 1) * T_C, c * O_C:(c + 1) * O_C] = \
            np.ascontiguousarray(res.results[core]["yt"].T)
    return out


def _sim_test(t_c=512, o_c=256, k=1024):
    """CoreSim numerics check at reduced size."""
    from concourse.bass_interp import CoreSim
    rng = np.random.default_rng(0)
    xv = rng.standard_normal((t_c, k), dtype=np.float32)
    wv = rng.standard_normal((o_c, k), dtype=np.float32) * 0.03
    mv = rng.integers(0, 2, size=(o_c, k)).astype(np.int32)
    bv = rng.standard_normal(o_c).astype(np.float32)

    nc = build_program(t_c=t_c, o_c=o_c, k=k, hw=False)
    sim = CoreSim(nc)
    sim.tensor("x")[:] = xv
    sim.tensor("w")[:] = wv
    sim.tensor("m")[:] = mv
    sim.tensor("b")[:] = bv
    sim.simulate(check_with_hw=False)
    got = np.array(sim.tensor("yt")).T  # [t_c, o_c]

    wm = wv * mv
    ref = xv @ wm.T + bv
    num = np.linalg.norm((got - ref).astype(np.float64))
    den = np.linalg.norm(ref.astype(np.float64)) + 1e-30
    print(f"sim rel err: {num / den:.6g}  (max abs {np.abs(got - ref).max():.4g})")
    assert num / den < 2e-2, "sim numerics check FAILED"
    print("SIM OK")


if __name__ == "__main__":
    _sim_test()
